# revision 12
# baseline (speedup 1.0000x reference)
"""Trainium2 Bass kernel for LocalDynamicGraph edge-feature construction.

Reference computation (per batch b, point n, neighbor slot k):
    out[b, n, c,      k] = x[b, idx[b,n,k], c] - x[b, n, c]   (c in [0,64))
    out[b, n, 64 + c, k] = x[b, n, c]
Output shape (B, N, 2C, K) = (8, 16384, 128, 20) float32.

Strategy: pure data parallel, one batch per NeuronCore (B == 8 cores).

Per-core cost is DMA-engine bound: the 168 MB output store (full-rate,
~466 us of the 16-engine pool) plus the SWDGE gather (328K descriptors of
256 B; sub-512 B descriptors run at half rate, so another ~466 us).  Both
terms are irreducible, so the kernel's job is to keep the DMA pool busy
from t=0 to the end:

  * Four tiny warm-up dma_gathers (one per SWDGE queue) are issued first so
    the ~75 us one-time Q7 ucode load overlaps the input preloads instead
    of stalling the first real gather (the old kernel idled 0-100 us).
  * x is uploaded once, host-wrapped to [p, u, t, c] so the whole 4 MB
    center image loads as one dense 32 KB/partition DMA.  The same DRAM
    buffer doubles as the gather pool: the host remaps each neighbor index
    m -> (m%128)*128 + m//128, which is exactly row (p, u, t) of the
    wrapped layout.
  * Each chunk's store is split in half: the diff half (gather-dependent)
    goes on the sync engine's HWDGE queue, while the center half -- which
    only needs x -- is produced by ACT and stored via the scalar engine's
    queue.  Center stores therefore flow from ~15 us on and fill the DMA
    pool while the gather ucode is still loading, with no head-of-line
    blocking between the two store streams.
  * 64 chunks of 256 points (J=2) with 4 gather buffers / 3 store buffers
    keep all four SWDGE queues and both store queues packed mid-stream;
    the final chunk is processed per-128-point tile to halve the drain
    tail.
"""

import sys

sys.path.insert(0, "/opt/trn_rl_repo")

import numpy as np

B, N, C, K = 8, 16384, 64, 20
P = 128          # SBUF partitions == points per point-tile
J = 2            # point-tiles per chunk
PTS = P * J      # points per chunk
NCHUNK = N // PTS
NIDX = PTS * K   # gather indices per chunk
IDX_COLS = NIDX // 16
M = 2 * C * K    # 2560 output elements per point
HALF = C * K     # 1280: diff half / center half

_compiled = None


def _build():
    import concourse.bacc as bacc
    import concourse.mybir as mybir
    import concourse.tile as tile
    import concourse.bass as bass
    from concourse._compat import get_trn_type

    nc = bacc.Bacc(
        get_trn_type() or "TRN2",
        target_bir_lowering=False,
        debug=True,
        num_swdge_queues=4,
    )
    # x wrapped to [p, (u t c)]: point n = u*PTS + t*128 + p lives at
    # partition p, free offset (u*J + t)*C.  Dense 32 KB per partition.
    xw_in = nc.dram_tensor(
        "xw", [P, NCHUNK * J * C], mybir.dt.float32, kind="ExternalInput"
    )
    idx_in = nc.dram_tensor(
        "idxw", [P, NCHUNK * IDX_COLS], mybir.dt.int16, kind="ExternalInput"
    )
    wz_in = nc.dram_tensor("wz", [P, 4 * 64], mybir.dt.int16, kind="ExternalInput")
    y_out = nc.dram_tensor("y", [N, M], mybir.dt.float32, kind="ExternalOutput")

    # Gather-pool view of the same buffer: row r = p*(NCHUNK*J) + u*J + t,
    # reached by host-remapped indices (m%128)*128 + m//128.
    xg = xw_in.rearrange("p (r c) -> (p r) c", c=C)
    y_r = y_out.rearrange("(u t p) m -> u p t m", t=J, p=P)

    with tile.TileContext(nc) as tc:
        with (
            tc.tile_pool(name="wp", bufs=1) as wp,
            tc.tile_pool(name="xp", bufs=1) as xp,
            tc.tile_pool(name="ip", bufs=1) as ip,
            tc.tile_pool(name="gp", bufs=8) as gp,
            tc.tile_pool(name="dp", bufs=2) as dp,
            tc.tile_pool(name="cp", bufs=2) as cp,
        ):
            # -- warm-up: one tiny gather per SWDGE queue kicks off the Q7
            # ucode load at t~0, overlapping the input DMAs below.
            WNI = 1024
            widx = wp.tile([P, 4 * (WNI // 16)], mybir.dt.int16)
            nc.scalar.dma_start(out=widx[:], in_=wz_in[:, :])
            wout = wp.tile([P, 4, WNI // 128, C], mybir.dt.float32)
            for q in range(4):
                nc.gpsimd.dma_gather(
                    wout[:, q], xg,
                    widx[:, q * (WNI // 16) : (q + 1) * (WNI // 16)],
                    WNI, WNI, C, single_packet=False, queue_num=q,
                )

            # -- inputs: x + first idx chunk on the scalar queue (needed
            # earliest, done ~13 us); the bulk idx image on the sync queue,
            # where it finishes long before the first diff store needs it.
            x_sb = xp.tile([P, NCHUNK, J, C], mybir.dt.float32)
            nc.scalar.dma_start(
                out=x_sb[:].rearrange("p u t c -> p (u t c)"), in_=xw_in[:, :]
            )
            idx_first = ip.tile([P, IDX_COLS], mybir.dt.int16)
            nc.scalar.dma_start(out=idx_first[:], in_=idx_in[:, :IDX_COLS])
            # bulk idx image in two stages: chunks 1-24 up front, the rest
            # queued behind the first diff store so the 4 MB x load (which
            # gates every center copy) finishes first.
            SPLIT = 24
            idx_rest = ip.tile([P, (NCHUNK - 1) * IDX_COLS], mybir.dt.int16)
            nc.sync.dma_start(
                out=idx_rest[:, : SPLIT * IDX_COLS],
                in_=idx_in[:, IDX_COLS : (SPLIT + 1) * IDX_COLS],
            )

            def ctr_bc_ap(u, t0, t1):
                c_ap = x_sb[:, u, t0:t1, :]
                return bass.AP(
                    c_ap.tensor, c_ap.offset, list(c_ap.ap) + [[0, K]]
                )

            for u in range(NCHUNK):
                idx_u = (
                    idx_first[:]
                    if u == 0
                    else idx_rest[:, (u - 1) * IDX_COLS : u * IDX_COLS]
                )
                g = gp.tile([P, J * K, C], mybir.dt.float32)
                last = u == NCHUNK - 1
                if not last:
                    if u < 4:
                        hc = IDX_COLS // 2
                        for h in range(2):
                            nc.gpsimd.dma_gather(
                                g[:, h * K : (h + 1) * K, :], xg,
                                idx_u[:, h * hc : (h + 1) * hc],
                                NIDX // 2, NIDX // 2, C,
                                single_packet=False, queue_num=u % 4,
                            )
                    else:
                        nc.gpsimd.dma_gather(
                            g[:], xg, idx_u, NIDX, NIDX, C,
                            single_packet=False, queue_num=u % 4,
                        )
                    dt_ = dp.tile([P, J, HALF], mybir.dt.float32)
                    cs = cp.tile([P, J, HALF], mybir.dt.float32)
                    nc.vector.tensor_sub(
                        dt_[:].rearrange("p t (c k) -> p t c k", c=C),
                        g[:].rearrange("p (t k) c -> p t c k", t=J),
                        ctr_bc_ap(u, 0, J),
                    )
                    if u < NCHUNK - 12:
                        # ACT produces center halves as fast as the store
                        # queue drains them (head-fill while gathers warm up)
                        nc.scalar.copy(
                            cs[:].rearrange("p t (c k) -> p t c k", c=C),
                            ctr_bc_ap(u, 0, J),
                        )
                    else:
                        # tail: pace the center stream at chunk rate (behind
                        # the sub on DVE) so center stores still overlap the
                        # final gathers instead of finishing early
                        nc.vector.tensor_copy(
                            cs[:].rearrange("p t (c k) -> p t c k", c=C),
                            ctr_bc_ap(u, 0, J),
                        )
                    nc.sync.dma_start(out=y_r[u, :, :, 0:HALF], in_=dt_[:])
                    if u == 0:
                        nc.sync.dma_start(
                            out=idx_rest[:, SPLIT * IDX_COLS :],
                            in_=idx_in[:, (SPLIT + 1) * IDX_COLS :],
                        )
                    nc.scalar.dma_start(out=y_r[u, :, :, HALF:M], in_=cs[:])
                else:
                    # final chunk: per-tile pipeline halves the drain tail.
                    # idx list order is (t, k, p), so the first half of the
                    # wrapped columns is exactly tile t=0.
                    dt_ = dp.tile([P, J, HALF], mybir.dt.float32)
                    cs = cp.tile([P, J, HALF], mybir.dt.float32)
                    hc = IDX_COLS // J
                    for t in range(J):
                        nc.gpsimd.dma_gather(
                            g[:, t * K : (t + 1) * K, :],
                            xg,
                            idx_u[:, t * hc : (t + 1) * hc],
                            NIDX // J,
                            NIDX // J,
                            C,
                            single_packet=False,
                            queue_num=(u + t) % 4,
                        )
                        nc.vector.tensor_sub(
                            dt_[:, t : t + 1].rearrange(
                                "p t (c k) -> p t c k", c=C
                            ),
                            g[:, t * K : (t + 1) * K, :].rearrange(
                                "p (t k) c -> p t c k", t=1
                            ),
                            ctr_bc_ap(u, t, t + 1),
                        )
                        nc.scalar.copy(
                            cs[:, t : t + 1].rearrange(
                                "p t (c k) -> p t c k", c=C
                            ),
                            ctr_bc_ap(u, t, t + 1),
                        )
                        nc.sync.dma_start(
                            out=y_r[u, :, t : t + 1, 0:HALF],
                            in_=dt_[:, t : t + 1],
                        )
                        nc.scalar.dma_start(
                            out=y_r[u, :, t : t + 1, HALF:M],
                            in_=cs[:, t : t + 1],
                        )

    nc.compile()
    return nc


def _wrap_x(x_b: np.ndarray) -> np.ndarray:
    """x (N, C) f32 -> (128, NCHUNK*J*C) wrapped image."""
    return np.ascontiguousarray(
        x_b.reshape(NCHUNK, J, P, C).transpose(2, 0, 1, 3).reshape(P, -1)
    )


def _wrap_indices(idx_b: np.ndarray) -> np.ndarray:
    """idx (N, K) int -> (128, NCHUNK*IDX_COLS) int16 SBUF image.

    Indices are remapped to the wrapped-x row order, then laid out so
    gather slot (t*K + k) of partition p holds neighbor k of point
    chunk_base + t*128 + p (SWDGE wrap: idx i at partition i%16, col i//16,
    tiled across the eight 16-partition bands).
    """
    m = np.asarray(idx_b)
    mp = (m % P) * (NCHUNK * J) + m // P
    blk = mp.reshape(NCHUNK, J, P, K)
    lin = blk.transpose(0, 1, 3, 2).reshape(NCHUNK, NIDX)
    wrapped = lin.reshape(NCHUNK, IDX_COLS, 16)
    img = wrapped.transpose(2, 0, 1).reshape(16, -1)
    return np.tile(img, (8, 1)).astype(np.int16)


def make_in_maps(x: np.ndarray, idx: np.ndarray) -> list[dict]:
    x = np.asarray(x, dtype=np.float32)
    idx = np.asarray(idx)
    wz = np.zeros((P, 4 * 64), dtype=np.int16)
    return [
        {"xw": _wrap_x(x[b]), "idxw": _wrap_indices(idx[b]), "wz": wz}
        for b in range(B)
    ]


def kernel(x: np.ndarray, idx: np.ndarray) -> np.ndarray:
    from concourse.bass_utils import run_bass_kernel_spmd

    global _compiled
    if _compiled is None:
        _compiled = _build()
    nc = _compiled

    res = run_bass_kernel_spmd(nc, make_in_maps(x, idx), core_ids=list(range(B)))
    out = np.stack([res.results[b]["y"].reshape(N, 2 * C, K) for b in range(B)])
    return out
